# revision 1
# baseline (speedup 1.0000x reference)
"""Int8Linear TRN2 kernel: y = x @ (W_int8 * scale)^T + bias.

Column-parallel across 8 NeuronCores: each core gets a [2048, 4096] shard
of W (as W^T, contiguous int8), the full x, and its bias slice.

Device strategy per core:
  - weights stream HBM->SBUF via SWDGE casting DMA (int8 -> bf16), so HBM
    traffic stays 1 B/weight and no compute engine does the dequant
  - x*scale is split hi/lo into two bf16 operands packed into the matmul
    stationary operand (M = 16 hi + 16 lo = 32), recovering ~fp32 accuracy
    at zero extra PE streaming cost
  - 4 PSUM banks accumulate o-groups of 512 over the 32 k-chunks
  - epilogue: y = psum_hi + bias + psum_lo on VectorE, DMA out fp32
"""

import os

import numpy as np

IN_F = 4096
OUT_F = 16384
NT = 16
NCORES = 8
O_PER = OUT_F // NCORES  # 2048
NCH = IN_F // 128  # 32 k-chunks
NG = O_PER // 512  # 4 o-groups

_CACHE = {}
LAST_EXEC_NS = None


def _install_drain_patch():
    """walrus codegen only allows 1 sem-wait per SP instruction; Tile's
    kernel-tail Drain aggregates many. Split them across sync nops."""
    from concourse.tile import TileContext
    from concourse.tile_scheduler import N_PROCS
    from concourse.vector_clock import VectorClock
    from bass_rust import ScopedClock

    if getattr(TileContext, "_drain_patched", False):
        return

    def _patched(self, tick_clock, wait_clock):
        gc = tick_clock.global_clock
        ticks = [gc[p] for p in range(N_PROCS)]
        for i in range(N_PROCS):
            partial = VectorClock(
                [ticks[p] if p == i else 0 for p in range(N_PROCS)]
            )
            if all(t == 0 for t in partial):
                continue
            nop = self.nc.sync.nop(hint="tail_wait", nofuse=True)
            wait_clock.add_sem_waits(nop.ins, ScopedClock({None: partial}))
        self.nc.sync.drain()
        self.nc.all_engine_barrier()
        assert self.sems is not None
        popped = self.nc._tile_sem_poison_stack.pop()
        assert popped is self._sem_poison
        self.nc.clear_and_free_semaphores(list(self.sems.allocated().values()))
        self.nc.all_engine_barrier()

    TileContext._drain_and_barrier = _patched
    TileContext._drain_patched = True


def _split_multi_waits(nc):
    """walrus codegen allows only one sem-wait per instruction: hoist all
    but the last wait of any instruction onto same-engine NoOps before it."""
    from concourse import mybir

    cnt = 0
    for fn in nc.m.functions:
        for bb in fn.blocks:
            out = []
            for inst in bb.instructions:
                si = inst.sync_info
                if si is not None and si.on_wait and len(si.on_wait) > 1:
                    waits = list(si.on_wait)
                    for w in waits[:-1]:
                        cnt += 1
                        nop = mybir.InstNoOp(
                            name=f"{inst.name}-sw{cnt}", ins=[], outs=[]
                        )
                        nop.engine = inst.engine
                        nop.sync_info = mybir.SyncInfo(on_wait=[w], on_update=[])
                        out.append(nop)
                    si.on_wait = [waits[-1]]
                out.append(inst)
            bb.instructions[:] = out


def _build_nc():
    import concourse.bass as bass
    import concourse.mybir as mybir
    from concourse.tile import TileContext

    _install_drain_patch()

    nc = bass.Bass(trn_type="TRN2")
    xt = nc.dram_tensor("xt", [128, NCH * 64], mybir.dt.bfloat16, kind="ExternalInput")
    wt = nc.dram_tensor("wt", [IN_F, O_PER], mybir.dt.int8, kind="ExternalInput")
    br = nc.dram_tensor("br", [NT, O_PER], mybir.dt.float32, kind="ExternalInput")
    y = nc.dram_tensor("y", [NT, O_PER], mybir.dt.float32, kind="ExternalOutput")

    with TileContext(nc) as tc:
        with (
            tc.tile_pool(name="xp", bufs=1) as xp,
            tc.tile_pool(name="bp", bufs=1) as bp,
            tc.tile_pool(name="wp", bufs=1) as wp,
            tc.tile_pool(name="pp", bufs=1, space="PSUM") as pp,
            tc.tile_pool(name="op", bufs=2) as op,
        ):
            xsb = xp.tile([128, NCH * 64], mybir.dt.bfloat16)
            nc.sync.dma_start(out=xsb[:], in_=xt[:])
            bsb = bp.tile([NT, O_PER], mybir.dt.float32)
            nc.sync.dma_start(out=bsb[:], in_=br[:])

            psums = [
                pp.tile([64, 512], mybir.dt.float32, tag=f"ps{g}", name=f"ps{g}") for g in range(NG)
            ]
            for n in range(NCH):
                wsb = wp.tile(
                    [128, O_PER], mybir.dt.bfloat16, tag=f"w{n}", name=f"w{n}"
                )
                nc.gpsimd.dma_start(
                    out=wsb[:], in_=wt[n * 128 : (n + 1) * 128, :]
                )
                for g in range(NG):
                    nc.tensor.matmul(
                        psums[g][:, :],
                        lhsT=xsb[:, n * 64 : (n + 1) * 64],
                        rhs=wsb[:, g * 512 : (g + 1) * 512],
                        start=(n == 0),
                        stop=(n == NCH - 1),
                    )
            for g in range(NG):
                osb = op.tile([NT, 512], mybir.dt.float32, tag="o")
                nc.vector.tensor_copy(osb[:], psums[g][0:NT, :])
                nc.vector.tensor_add(
                    osb[:], osb[:], bsb[:, g * 512 : (g + 1) * 512]
                )
                nc.vector.tensor_add(osb[:], osb[:], psums[g][32:48, :])
                nc.sync.dma_start(out=y[:, g * 512 : (g + 1) * 512], in_=osb[:])
    _split_multi_waits(nc)
    return nc


def kernel(x, weight_int8, weight_scale, bias):
    global LAST_EXEC_NS
    import ml_dtypes
    from concourse.bass_utils import run_bass_kernel_spmd

    x = np.asarray(x, dtype=np.float32)
    w = np.asarray(weight_int8)
    if w.dtype != np.int8:
        w = w.astype(np.int8)
    scale = float(np.asarray(weight_scale, dtype=np.float32))
    bias = np.asarray(bias, dtype=np.float32)

    # hi/lo bf16 split of x*scale, packed to the SBUF stationary layout:
    # xt_host[p, n*32 + m] = xs_T[n*128 + p, m], cols m: 0..15 hi, 16..31 lo
    xs = x * np.float32(scale)
    hi = xs.astype(ml_dtypes.bfloat16)
    lo = (xs - hi.astype(np.float32)).astype(ml_dtypes.bfloat16)
    xtf = np.zeros((IN_F, 64), dtype=ml_dtypes.bfloat16)
    xtf[:, 0:16] = hi.T
    xtf[:, 32:48] = lo.T
    xt_host = np.ascontiguousarray(
        xtf.reshape(NCH, 128, 64).transpose(1, 0, 2).reshape(128, NCH * 64)
    )

    if "nc" not in _CACHE:
        _CACHE["nc"] = _build_nc()
    nc = _CACHE["nc"]

    in_maps = []
    for c in range(NCORES):
        wshard = w[c * O_PER : (c + 1) * O_PER, :]  # [2048, 4096]
        wt_c = np.ascontiguousarray(wshard.T)  # [4096, 2048] int8
        b_c = np.ascontiguousarray(
            np.broadcast_to(bias[c * O_PER : (c + 1) * O_PER][None, :], (NT, O_PER))
        )
        in_maps.append({"xt": xt_host, "wt": wt_c, "br": b_c})

    trace = bool(os.environ.get("BASS_KERNEL_TRACE"))
    br = run_bass_kernel_spmd(nc, in_maps, list(range(NCORES)), trace=trace)
    LAST_EXEC_NS = br.exec_time_ns
    return np.concatenate([br.results[c]["y"] for c in range(NCORES)], axis=1)

